# revision 66
# baseline (speedup 1.0000x reference)
"""DGCNN forward on 8 Trainium2 NeuronCores, data-parallel over batch.

Per core (one point cloud, x [3, 2048]):
  4 edge-conv blocks, each:
    s[n,m] = 2*x_n.x_m - |x_m|^2            (fp32 PE matmul; row-constant
                                             -|x_n|^2 dropped: rank-invariant per row)
    exact top-20 of each s row: 3 rounds of (max8, max_index, match_replace) on DVE
    x' = lrelu(max_k A[:, idx_k] + B)       (A = Wn@x, B = (Wc-Wn)@x; edge features
                                             never materialized: conv is linear and
                                             lrelu/max commute)
  then g = lrelu(Wg @ concat(x1..x4)), out = max_n g.

Schedule: 3-stage software pipeline per block (gram+copies one chunk ahead,
topk+gather current, reduce/tail two chunks behind) so the in-order engine
sequencers never head-block on the indirect gather DMAs (994 ns fixed
SWDGE overhead each, the Pool-engine pacer).  The rank-1 neighbor is always
the point itself, so gather slot 0 is a plain contiguous DMA with no topk
dependency (19 indirect gathers per chunk, not 20, issued in 3 waves right
after each max_index round).  Each block's per-column prep (2x,
sum-of-squares row at psum partition 64, next A^T table chunk) is fused
into the previous block's tail stage; the global conv runs interleaved
with block 4 in column pieces as they complete.

All matmuls exact fp32 (fp32r's ~1e-3 noise corrupts the knn sets).
"""

import numpy as np
from contextlib import ExitStack

import concourse.bass as bass
import concourse.bacc as bacc
import concourse.mybir as mybir
from concourse.bass import IndirectOffsetOnAxis
from concourse.tile import TileContext

F32 = mybir.dt.float32
U16 = mybir.dt.uint16
U32 = mybir.dt.uint32

B, N, KNN, P = 8, 2048, 20, 128
NCHUNK = N // P            # 16
NEG = -3.0e38
SLOPE = 0.2
BLOCKS = [(3, 64), (64, 64), (64, 128), (128, 256)]
ACT = mybir.ActivationFunctionType


def build_nc():
    nc = bacc.Bacc("TRN2", target_bir_lowering=False)

    x_in = nc.dram_tensor("x", [3, N], F32, kind="ExternalInput")
    w_in = {}
    for bi, (c, o) in enumerate(BLOCKS):
        w_in[f"wnt{bi}"] = nc.dram_tensor(f"wnt{bi}", [c, o], F32, kind="ExternalInput")
        w_in[f"wdt{bi}"] = nc.dram_tensor(f"wdt{bi}", [c, o], F32, kind="ExternalInput")
    w_in["wgt"] = nc.dram_tensor("wgt", [512, 1024], F32, kind="ExternalInput")
    id_in = nc.dram_tensor("ident", [P, P], F32, kind="ExternalInput")
    out_d = nc.dram_tensor("out", [1024, 1], F32, kind="ExternalOutput")

    # DRAM scratch: per-block A^T feature tables
    at_dram = [
        nc.dram_tensor(f"at{bi}_scratch", [N, o], F32, kind="Internal")
        for bi, (_, o) in enumerate(BLOCKS)
    ]

    with TileContext(nc) as tc, ExitStack() as ctx:
        ep = ctx.enter_context
        const = ep(tc.tile_pool(name="const", bufs=1))
        wpool = ep(tc.tile_pool(name="weights", bufs=1))
        xpool = ep(tc.tile_pool(name="xtiles", bufs=1))
        spool = ep(tc.tile_pool(name="sbuf", bufs=2))
        tkpool = ep(tc.tile_pool(name="topk", bufs=2))
        gpool = ep(tc.tile_pool(name="gather", bufs=3))
        pp_s = ep(tc.tile_pool(name="ps_s", bufs=4, space="PSUM"))   # [128,512] -> 4 banks
        pp_m = ep(tc.tile_pool(name="ps_m", bufs=2, space="PSUM"))   # small tiles

        # ---- input x first: everything on the critical path hangs off it ----
        gl1 = xpool.tile([33, N], F32)
        nc.vector.memset(gl1[:], 0.0)
        nc.sync.dma_start(out=gl1[0:3, :], in_=x_in[:])

        # ---- constants + small per-block weights ----
        ident = const.tile([P, P], F32)
        nc.sync.dma_start(out=ident[:], in_=id_in[:])
        ones_col = const.tile([P, 1], F32)
        nc.vector.memset(ones_col[:], 1.0)
        ones_row = const.tile([1, P], F32)
        nc.vector.memset(ones_row[:], 1.0)
        # PE p-state warmup: ~3 us of continuous execution brings the clock
        # to full speed before the first real matmuls on the startup chain
        warm = pp_m.tile([1, P], F32, space="PSUM", tag="mm", name="warm")
        for _ in range(8):
            nc.tensor.matmul(out=warm[:], lhsT=ones_col[:1, :], rhs=ones_row[:],
                             start=True, stop=True)
        wnT, wdT = [], []
        for bi, (c, o) in enumerate(BLOCKS):
            wn = wpool.tile([c, o], F32, tag=f"wn{bi}")
            nc.sync.dma_start(out=wn[:], in_=w_in[f"wnt{bi}"][:])
            wd = wpool.tile([c, o], F32, tag=f"wd{bi}")
            nc.sync.dma_start(out=wd[:], in_=w_in[f"wdt{bi}"][:])
            wnT.append(wn)
            wdT.append(wd)
        # big global-conv weights last (not needed until block 4)
        wgT = [wpool.tile([P, 1024], F32, tag=f"wg{k}", name=f"wgT{k}")
               for k in range(4)]
        for k in range(4):
            # Act-engine queue: keeps 2 MB of loads out of SP's queue, which
            # carries the block-1 A-table writes the first gathers wait on
            nc.scalar.dma_start(out=wgT[k][:], in_=w_in["wgt"][k * P:(k + 1) * P, :])
        # k=0 rows 64:128 again at base partition 0 (x2t half-contraction)
        wg0b = wpool.tile([64, 1024], F32)
        nc.scalar.dma_start(out=wg0b[:], in_=w_in["wgt"][64:128, :])

        # ---- persistent feature tiles ----
        # x1t/x2t rows 0:64 = x1/x2, row 64 = ones (fused gram lhs for blocks 2,3)
        x1t = xpool.tile([65, N], F32)
        x2t = xpool.tile([65, N], F32)
        c3 = xpool.tile([P, N], F32)      # x3
        c4a = xpool.tile([P, N], F32)     # x4 rows 0:128
        c4b = xpool.tile([P, N], F32)     # x4 rows 128:256
        gr2 = xpool.tile([65, N], F32)    # rows 0:64 = 2*x1, row 64 = -|x1|^2
        gr3 = xpool.tile([65, N], F32)    # rows 0:64 = 2*x2, row 64 = -|x2|^2
        x24 = xpool.tile([P, N], F32)     # 2*x3
        negxx4 = xpool.tile([1, N], F32)  # -|x3|^2
        redv = xpool.tile([P, 48], F32)   # global-conv per-(oc,piece) maxima
        nc.vector.memset(x1t[64:65, :], 1.0)
        nc.vector.memset(x2t[64:65, :], 1.0)

        # ---- block-1 prep (from the input x0) ----
        # augmented gram operands: gl1 = [x0; 0-pad; ones], gr1 = [2x0; 0-pad; -|x0|^2]
        gr1 = xpool.tile([33, N], F32)
        nc.vector.memset(gr1[:], 0.0)
        nc.vector.memset(gl1[32:33, :], 1.0)
        nc.scalar.activation(out=gr1[0:3, :], in_=gl1[0:3, :], func=ACT.Copy, scale=2.0)
        for q in range(4):
            sqq = spool.tile([3, 512], F32, tag="sqq", name=f"sqq{q}")
            nc.scalar.activation(out=sqq[:], in_=gl1[0:3, q * 512:(q + 1) * 512],
                                 func=ACT.Square)
            mq = pp_m.tile([33, 512], F32, space="PSUM", tag="tp")
            nc.tensor.matmul(out=mq[32:33, :], lhsT=ones_col[:3, :], rhs=sqq[:],
                             start=True, stop=True)
            # matmul lands the row at psum partition 32: the Act copy writes
            # gr1 row 32 directly, no DMA hop before the first gram
            nc.scalar.activation(out=gr1[32:33, q * 512:(q + 1) * 512],
                                 in_=mq[32:33, :], func=ACT.Copy, scale=-1.0)
        def emit_at0_prep():
            # emitted after the first gram so DVE reaches the scans sooner;
            # the table is only needed by the first gather wave
            for i in range(NCHUNK):
                pa = pp_m.tile([P, 64], F32, space="PSUM", tag="mm")
                nc.tensor.matmul(out=pa[:], lhsT=gl1[0:3, i * P:(i + 1) * P],
                                 rhs=wnT[0][:], start=True, stop=True)
                at_sb = spool.tile([P, 64], F32, tag="at_sb")
                nc.scalar.copy(out=at_sb[:], in_=pa[:])
                nc.sync.dma_start(out=at_dram[0][i * P:(i + 1) * P, :], in_=at_sb[:])

        # per-block gram operand config
        #   fused: (lhs_tile, lhs_rows, rhs_tile)     two-matmul: (lhs, C, rhs2x, negxx)
        gram_cfg = [
            ("fused", gl1, 33, gr1, None),
            ("fused", x1t, 65, gr2, None),
            ("fused", x2t, 65, gr3, None),
            ("two", c3, P, x24, negxx4),
        ]
        # B-stage destinations + next-block prep config per block
        #   (dsts, nxt_main, nxt_gr2x, nxt_negxx_row_tile_or_None, nxt_at_lhs, nxt_wn)
        tail_cfg = [
            ([x1t], gr2, True, wnT[1]),
            ([x2t], gr3, True, wnT[2]),
            ([c3], None, False, wnT[3]),   # writes x24/negxx4 directly
            ([c4a, c4b], None, False, None),
        ]
        xcat = [None, c3, c4a, c4b]       # k=0 handled as two half-contractions

        def emit_final_oc(oc):
            red1 = spool.tile([P, 1], F32, tag="red1")
            nc.vector.tensor_reduce(out=red1[:], in_=redv[:, oc * 6:(oc + 1) * 6],
                                    axis=mybir.AxisListType.X, op=mybir.AluOpType.max)
            reds = spool.tile([P, 1], F32, tag="reds")
            nc.vector.tensor_scalar_mul(reds[:], red1[:], SLOPE)
            nc.vector.tensor_tensor(out=red1[:], in0=red1[:], in1=reds[:],
                                    op=mybir.AluOpType.max)
            nc.sync.dma_start(out=out_d[oc * P:(oc + 1) * P, :], in_=red1[:])

        st = {}   # keyed (bi, i)

        def emit_a1(bi2, i, qs=(0, 1, 2, 3)):
            mode, glhs, grows, grhs, gneg = gram_cfg[bi2]
            key = (bi2, i)
            if key not in st:
                st[key] = {"s_sb": tkpool.tile([P, N], F32, tag="s_sb",
                                               name=f"s_sb_{bi2}_{i}")}
            s_sb = st[key]["s_sb"]
            for q in qs:
                sl = slice(q * 512, (q + 1) * 512)
                ph = pp_s.tile([P, 512], F32, space="PSUM", tag="ph")
                if mode == "fused":
                    nc.tensor.matmul(out=ph[:], lhsT=glhs[:grows, i * P:(i + 1) * P],
                                     rhs=grhs[:, sl], start=True, stop=True)
                else:
                    nc.tensor.matmul(out=ph[:], lhsT=glhs[:grows, i * P:(i + 1) * P],
                                     rhs=grhs[:, sl], start=True, stop=False)
                    nc.tensor.matmul(out=ph[:], lhsT=ones_row[:, :P],
                                     rhs=gneg[:, sl], start=False, stop=True)
                nc.scalar.copy(out=s_sb[:, sl], in_=ph[:])

        for bi, (C, O) in enumerate(BLOCKS):
            gq = []   # pending global-conv (slot, cols, oc) jobs

            def emit_a2(i, bi=bi, O=O):
                # topk rounds interleaved with the gather waves so the Pool
                # engine starts each chunk's gathers ~4.5 us earlier
                s_sb = st[(bi, i)]["s_sb"]
                v8 = tkpool.tile([P, 8], F32, tag="v8")
                # one index tile per wave: a shared tile would WAR-chain the
                # next round's max_index behind this wave's gather DMAs
                iw = [gpool.tile([P, 8], U32, tag=f"iw{r}", name=f"iw{r}")
                      for r in range(3)]
                gath = gpool.tile([P, KNN, O], F32, tag="gath")
                # rank-1 is always the point itself (s[p,p]-s[p,m]=|x_p-x_m|^2
                # >= 0; near-ties are feature-duplicates with equal A rows), so
                # slot 0 is a contiguous chunk read off the Pool engine, with
                # no dependency on the scans at all
                nc.sync.dma_start(out=gath[:, 0, :],
                                  in_=at_dram[bi][i * P:(i + 1) * P, :])
                waves = [(1, 1, 7), (8, 0, 8), (16, 0, 4)]
                for r, (j0, w0, nj) in enumerate(waves):
                    if r == 0 and i == 0:
                        # block start: scan the columns that landed early
                        # (pre-emitted gram quarters) before the last quarter
                        # arrives; top-8 of the halves' top-8s is exact
                        va = tkpool.tile([P, 16], F32, tag="va")
                        nc.vector.max(out=va[:, 0:8], in_=s_sb[:, 0:1536])
                        nc.vector.max(out=va[:, 8:16], in_=s_sb[:, 1536:2048])
                        nc.vector.max(out=v8[:], in_=va[:])
                    else:
                        nc.vector.max(out=v8[:], in_=s_sb[:])
                    nc.vector.max_index(out=iw[r][:], in_max=v8[:], in_values=s_sb[:])
                    for j in range(nj):
                        nc.gpsimd.indirect_dma_start(
                            out=gath[:, j0 + j, :], out_offset=None,
                            in_=at_dram[bi][:],
                            in_offset=IndirectOffsetOnAxis(
                                ap=iw[r][:, w0 + j:w0 + j + 1], axis=0))
                    if r < 2:
                        nc.vector.match_replace(out=s_sb[:], in_to_replace=v8[:],
                                                in_values=s_sb[:], imm_value=NEG)
                st[(bi, i)]["gath"] = gath

            def emit_b(i, bi=bi, C=C, O=O):
                gath = st[(bi, i)].pop("gath")
                mx = spool.tile([P, O], F32, tag="mx")
                if i >= NCHUNK - 2 or i <= 1:
                    # fill/drain chunks: reduce waves A+B first (they complete
                    # earlier), so only the small wave-C part sits behind the
                    # last DMAs while Pool works through its backlog
                    mab = spool.tile([P, O], F32, tag="xs", name="mab")
                    nc.vector.tensor_reduce(
                        out=mab[:], in_=gath[:, :16, :].rearrange("p k o -> p o k"),
                        axis=mybir.AxisListType.X, op=mybir.AluOpType.max)
                    nc.vector.tensor_reduce(
                        out=mx[:], in_=gath[:, 16:, :].rearrange("p k o -> p o k"),
                        axis=mybir.AxisListType.X, op=mybir.AluOpType.max)
                    nc.vector.tensor_tensor(out=mx[:], in0=mx[:], in1=mab[:],
                                            op=mybir.AluOpType.max)
                else:
                    nc.vector.tensor_reduce(
                        out=mx[:], in_=gath[:].rearrange("p k o -> p o k"),
                        axis=mybir.AxisListType.X, op=mybir.AluOpType.max)
                bt = pp_m.tile([P, O], F32, space="PSUM", tag="mm")
                nc.tensor.matmul(out=bt[:], lhsT=gram_cfg[bi][1][:C, i * P:(i + 1) * P],
                                 rhs=wdT[bi][:], start=True, stop=True)
                xt = spool.tile([P, O], F32, tag="xt")
                nc.vector.tensor_add(out=xt[:], in0=mx[:], in1=bt[:])
                xs = spool.tile([P, O], F32, tag="xs")
                nc.vector.tensor_scalar_mul(xs[:], xt[:], SLOPE)
                nc.vector.tensor_tensor(out=xt[:], in0=xt[:], in1=xs[:],
                                        op=mybir.AluOpType.max)

                dsts, nxt_gr, gr_dma, nxt_wn = tail_cfg[bi]
                cols = slice(i * P, (i + 1) * P)
                for q in range((O + P - 1) // P):
                    osz = min(P, O - q * P)
                    tp = pp_m.tile([P, P], F32, space="PSUM", tag="tp")
                    nc.tensor.transpose(out=tp[:osz, :], in_=xt[:, q * P:q * P + osz],
                                        identity=ident[:])
                    nc.scalar.copy(out=dsts[q][0:osz, cols], in_=tp[:osz, :])
                    if bi <= 2:
                        # next-block prep, fused on this chunk's columns
                        tgt2x = nxt_gr[0:osz, cols] if bi <= 1 else x24[0:osz, cols]
                        nc.scalar.activation(out=tgt2x, in_=tp[:osz, :],
                                             func=ACT.Copy, scale=2.0)
                        sqt = spool.tile([O, P], F32, tag="sqt")
                        nc.scalar.activation(out=sqt[:osz, :], in_=tp[:osz, :],
                                             func=ACT.Square)
                if bi <= 2:
                    # write the row-sum at psum partition 64 so the Act copy
                    # lands it directly in gr{2,3} row 64 (no DMA hop on the
                    # block-transition critical chain)
                    pn = pp_m.tile([65, P], F32, space="PSUM", tag="tp")
                    row = 64 if bi <= 1 else 0
                    nc.tensor.matmul(out=pn[row:row + 1, :], lhsT=ones_col[:O, :],
                                     rhs=sqt[:], start=True, stop=True)
                    if bi <= 1:
                        nc.scalar.activation(out=nxt_gr[64:65, cols],
                                             in_=pn[64:65, :], func=ACT.Copy,
                                             scale=-1.0)
                    else:
                        nc.scalar.activation(out=negxx4[:, cols], in_=pn[0:1, :],
                                             func=ACT.Copy, scale=-1.0)
                    # next-block A^T table chunk
                    O2 = BLOCKS[bi + 1][1]
                    pa = pp_m.tile([P, O2], F32, space="PSUM", tag="mm")
                    nc.tensor.matmul(out=pa[:], lhsT=dsts[0][0:O, cols], rhs=nxt_wn[:],
                                     start=True, stop=True)
                    at_sb = spool.tile([P, O2], F32, tag="at_sb")
                    nc.scalar.copy(out=at_sb[:], in_=pa[:])
                    nc.sync.dma_start(out=at_dram[bi + 1][cols, :], in_=at_sb[:])
                if bi == 3:
                    # global-conv pieces as their columns complete, dripped a
                    # few ocs per tail stage: a full 8-oc burst is ~34 us of
                    # PE matmuls that would head-block the next chunk's gram
                    if i in (3, 7, 11):
                        nq = i // 4
                        gq.extend((nq, slice(512 * nq, 512 * (nq + 1)), oc)
                                  for oc in range(8))
                    elif i == 13:
                        gq.extend((3, slice(1536, 1792), oc) for oc in range(8))
                    elif i == 14:
                        gq.extend((4, slice(1792, 1920), oc) for oc in range(8))
                    elif i == 15:
                        gq.extend((5, slice(1920, 2048), oc) for oc in range(8))
                    ndrain = 2 if i < 13 else len(gq)
                    for slot, ncols, oc in gq[:ndrain]:
                        emit_gpiece_oc(slot, ncols, oc)
                        if slot == 5:
                            emit_final_oc(oc)
                    del gq[:ndrain]

            def emit_gpiece_oc(slot, ncols, oc):
                nn = ncols.stop - ncols.start
                ocs = slice(oc * P, (oc + 1) * P)
                pg = pp_s.tile([P, 512], F32, space="PSUM", tag="ph")
                nc.tensor.matmul(out=pg[:, :nn], lhsT=wgT[0][0:64, ocs],
                                 rhs=x1t[0:64, ncols], start=True, stop=False)
                nc.tensor.matmul(out=pg[:, :nn], lhsT=wg0b[:, ocs],
                                 rhs=x2t[0:64, ncols], start=False, stop=False)
                for k in range(1, 4):
                    nc.tensor.matmul(out=pg[:, :nn], lhsT=wgT[k][:, ocs],
                                     rhs=xcat[k][:, ncols],
                                     start=False, stop=(k == 3))
                nc.vector.tensor_reduce(out=redv[:, oc * 6 + slot:oc * 6 + slot + 1],
                                        in_=pg[:, :nn], axis=mybir.AxisListType.X,
                                        op=mybir.AluOpType.max)

            if bi == 0:
                emit_a1(0, 0)
                emit_at0_prep()
            else:
                emit_a1(bi, 0, qs=(3,))   # quarters 0-2 pre-emitted by block bi-1
            for i in range(NCHUNK):
                if i + 1 < NCHUNK:
                    emit_a1(bi, i + 1)
                    if i + 1 == NCHUNK - 1 and bi < 3:
                        # next block's first gram, for the columns already
                        # complete (quarters 0-2 <- chunks 0..11): keeps PE fed
                        # through the gather drain at the block boundary
                        emit_a1(bi + 1, 0, qs=(0, 1, 2))
                emit_a2(i)
                # B(0) deferred one extra iteration: during the pipeline fill
                # Pool is still backlogged, and the wait before reduce(0)
                # would head-block the chunk-3 scans on the in-order DVE
                if i == 3:
                    emit_b(0)
                elif i == 4:
                    emit_b(1)
                    emit_b(2)
                elif i >= 5:
                    emit_b(i - 2)
            emit_b(NCHUNK - 2)
            emit_b(NCHUNK - 1)



    nc.compile()
    return nc


_NC_CACHE = None


def _get_nc():
    global _NC_CACHE
    if _NC_CACHE is None:
        _NC_CACHE = build_nc()
    return _NC_CACHE


def host_weights(W1, W2, W3, W4, Wg):
    ws = {}
    for bi, (wm, (c, o)) in enumerate(zip([W1, W2, W3, W4], BLOCKS)):
        wm = np.asarray(wm, dtype=np.float32)
        wn = wm[:, :c]
        wd = wm[:, c:] - wn
        ws[f"wnt{bi}"] = np.ascontiguousarray(wn.T)
        ws[f"wdt{bi}"] = np.ascontiguousarray(wd.T)
    ws["wgt"] = np.ascontiguousarray(np.asarray(Wg, dtype=np.float32).T)
    ws["ident"] = np.eye(P, dtype=np.float32)
    return ws


def kernel(x, W1, W2, W3, W4, Wg):
    from concourse.bass_utils import run_bass_kernel_spmd

    nc = _get_nc()
    x = np.asarray(x, dtype=np.float32)
    ws = host_weights(W1, W2, W3, W4, Wg)
    in_maps = [{"x": np.ascontiguousarray(x[b]), **ws} for b in range(B)]
    res = run_bass_kernel_spmd(nc, in_maps, core_ids=list(range(B)))
    outs = res.results if hasattr(res, "results") else res
    return np.stack([outs[b]["out"].reshape(1024) for b in range(B)], axis=0)


# revision 67
# speedup vs baseline: 1.0016x; 1.0016x over previous
"""DGCNN forward on 8 Trainium2 NeuronCores, data-parallel over batch.

Per core (one point cloud, x [3, 2048]):
  4 edge-conv blocks, each:
    s[n,m] = 2*x_n.x_m - |x_m|^2            (fp32 PE matmul; row-constant
                                             -|x_n|^2 dropped: rank-invariant per row)
    exact top-20 of each s row: 3 rounds of (max8, max_index, match_replace) on DVE
    x' = lrelu(max_k A[:, idx_k] + B)       (A = Wn@x, B = (Wc-Wn)@x; edge features
                                             never materialized: conv is linear and
                                             lrelu/max commute)
  then g = lrelu(Wg @ concat(x1..x4)), out = max_n g.

Schedule: 3-stage software pipeline per block (gram+copies one chunk ahead,
topk+gather current, reduce/tail two chunks behind) so the in-order engine
sequencers never head-block on the indirect gather DMAs (994 ns fixed
SWDGE overhead each, the Pool-engine pacer).  The rank-1 neighbor is always
the point itself, so gather slot 0 is a plain contiguous DMA with no topk
dependency (19 indirect gathers per chunk, not 20, issued in 3 waves right
after each max_index round).  Each block's per-column prep (2x,
sum-of-squares row at psum partition 64, next A^T table chunk) is fused
into the previous block's tail stage; the global conv runs interleaved
with block 4 in column pieces as they complete.

All matmuls exact fp32 (fp32r's ~1e-3 noise corrupts the knn sets).
"""

import numpy as np
from contextlib import ExitStack

import concourse.bass as bass
import concourse.bacc as bacc
import concourse.mybir as mybir
from concourse.bass import IndirectOffsetOnAxis
from concourse.tile import TileContext

F32 = mybir.dt.float32
U16 = mybir.dt.uint16
U32 = mybir.dt.uint32

B, N, KNN, P = 8, 2048, 20, 128
NCHUNK = N // P            # 16
NEG = -3.0e38
SLOPE = 0.2
BLOCKS = [(3, 64), (64, 64), (64, 128), (128, 256)]
ACT = mybir.ActivationFunctionType


def build_nc():
    nc = bacc.Bacc("TRN2", target_bir_lowering=False)

    x_in = nc.dram_tensor("x", [3, N], F32, kind="ExternalInput")
    w_in = {}
    for bi, (c, o) in enumerate(BLOCKS):
        w_in[f"wnt{bi}"] = nc.dram_tensor(f"wnt{bi}", [c, o], F32, kind="ExternalInput")
        w_in[f"wdt{bi}"] = nc.dram_tensor(f"wdt{bi}", [c, o], F32, kind="ExternalInput")
    w_in["wgt"] = nc.dram_tensor("wgt", [512, 1024], F32, kind="ExternalInput")
    id_in = nc.dram_tensor("ident", [P, P], F32, kind="ExternalInput")
    out_d = nc.dram_tensor("out", [1024, 1], F32, kind="ExternalOutput")

    # DRAM scratch: per-block A^T feature tables
    at_dram = [
        nc.dram_tensor(f"at{bi}_scratch", [N, o], F32, kind="Internal")
        for bi, (_, o) in enumerate(BLOCKS)
    ]

    with TileContext(nc) as tc, ExitStack() as ctx:
        ep = ctx.enter_context
        const = ep(tc.tile_pool(name="const", bufs=1))
        wpool = ep(tc.tile_pool(name="weights", bufs=1))
        xpool = ep(tc.tile_pool(name="xtiles", bufs=1))
        spool = ep(tc.tile_pool(name="sbuf", bufs=2))
        tkpool = ep(tc.tile_pool(name="topk", bufs=2))
        gpool = ep(tc.tile_pool(name="gather", bufs=3))
        pp_s = ep(tc.tile_pool(name="ps_s", bufs=4, space="PSUM"))   # [128,512] -> 4 banks
        pp_m = ep(tc.tile_pool(name="ps_m", bufs=2, space="PSUM"))   # small tiles

        # ---- input x first: everything on the critical path hangs off it ----
        gl1 = xpool.tile([33, N], F32)
        nc.vector.memset(gl1[:], 0.0)
        nc.sync.dma_start(out=gl1[0:3, :], in_=x_in[:])

        # ---- constants + small per-block weights ----
        ident = const.tile([P, P], F32)
        nc.sync.dma_start(out=ident[:], in_=id_in[:])
        ones_col = const.tile([P, 1], F32)
        nc.vector.memset(ones_col[:], 1.0)
        ones_row = const.tile([1, P], F32)
        nc.vector.memset(ones_row[:], 1.0)
        # PE p-state warmup: ~3 us of continuous execution brings the clock
        # to full speed before the first real matmuls on the startup chain
        warm = pp_m.tile([1, P], F32, space="PSUM", tag="mm", name="warm")
        for _ in range(8):
            nc.tensor.matmul(out=warm[:], lhsT=ones_col[:1, :], rhs=ones_row[:],
                             start=True, stop=True)
        wnT, wdT = [], []
        for bi, (c, o) in enumerate(BLOCKS):
            wn = wpool.tile([c, o], F32, tag=f"wn{bi}")
            nc.sync.dma_start(out=wn[:], in_=w_in[f"wnt{bi}"][:])
            wd = wpool.tile([c, o], F32, tag=f"wd{bi}")
            nc.sync.dma_start(out=wd[:], in_=w_in[f"wdt{bi}"][:])
            wnT.append(wn)
            wdT.append(wd)
        # big global-conv weights last (not needed until block 4)
        wgT = [wpool.tile([P, 1024], F32, tag=f"wg{k}", name=f"wgT{k}")
               for k in range(4)]
        for k in range(4):
            # Act-engine queue: keeps 2 MB of loads out of SP's queue, which
            # carries the block-1 A-table writes the first gathers wait on
            nc.scalar.dma_start(out=wgT[k][:], in_=w_in["wgt"][k * P:(k + 1) * P, :])
        # k=0 rows 64:128 again at base partition 0 (x2t half-contraction)
        wg0b = wpool.tile([64, 1024], F32)
        nc.scalar.dma_start(out=wg0b[:], in_=w_in["wgt"][64:128, :])

        # ---- persistent feature tiles ----
        # x1t/x2t rows 0:64 = x1/x2, row 64 = ones (fused gram lhs for blocks 2,3)
        x1t = xpool.tile([65, N], F32)
        x2t = xpool.tile([65, N], F32)
        c3 = xpool.tile([P, N], F32)      # x3
        c4a = xpool.tile([P, N], F32)     # x4 rows 0:128
        c4b = xpool.tile([P, N], F32)     # x4 rows 128:256
        gr2 = xpool.tile([65, N], F32)    # rows 0:64 = 2*x1, row 64 = -|x1|^2
        gr3 = xpool.tile([65, N], F32)    # rows 0:64 = 2*x2, row 64 = -|x2|^2
        x24 = xpool.tile([P, N], F32)     # 2*x3
        negxx4 = xpool.tile([1, N], F32)  # -|x3|^2
        redv = xpool.tile([P, 48], F32)   # global-conv per-(oc,piece) maxima
        nc.vector.memset(x1t[64:65, :], 1.0)
        nc.vector.memset(x2t[64:65, :], 1.0)

        # ---- block-1 prep (from the input x0) ----
        # augmented gram operands: gl1 = [x0; 0-pad; ones], gr1 = [2x0; 0-pad; -|x0|^2]
        gr1 = xpool.tile([33, N], F32)
        nc.vector.memset(gr1[:], 0.0)
        nc.vector.memset(gl1[32:33, :], 1.0)
        nc.scalar.activation(out=gr1[0:3, :], in_=gl1[0:3, :], func=ACT.Copy, scale=2.0)
        for q in range(4):
            sqq = spool.tile([3, 512], F32, tag="sqq", name=f"sqq{q}")
            nc.scalar.activation(out=sqq[:], in_=gl1[0:3, q * 512:(q + 1) * 512],
                                 func=ACT.Square)
            mq = pp_m.tile([33, 512], F32, space="PSUM", tag="tp")
            nc.tensor.matmul(out=mq[32:33, :], lhsT=ones_col[:3, :], rhs=sqq[:],
                             start=True, stop=True)
            # matmul lands the row at psum partition 32: the Act copy writes
            # gr1 row 32 directly, no DMA hop before the first gram
            nc.scalar.activation(out=gr1[32:33, q * 512:(q + 1) * 512],
                                 in_=mq[32:33, :], func=ACT.Copy, scale=-1.0)
        def emit_at0_prep():
            # emitted after the first gram so DVE reaches the scans sooner;
            # the table is only needed by the first gather wave
            for i in range(NCHUNK):
                pa = pp_m.tile([P, 64], F32, space="PSUM", tag="mm")
                nc.tensor.matmul(out=pa[:], lhsT=gl1[0:3, i * P:(i + 1) * P],
                                 rhs=wnT[0][:], start=True, stop=True)
                at_sb = spool.tile([P, 64], F32, tag="at_sb")
                nc.scalar.copy(out=at_sb[:], in_=pa[:])
                nc.sync.dma_start(out=at_dram[0][i * P:(i + 1) * P, :], in_=at_sb[:])

        # per-block gram operand config
        #   fused: (lhs_tile, lhs_rows, rhs_tile)     two-matmul: (lhs, C, rhs2x, negxx)
        gram_cfg = [
            ("fused", gl1, 33, gr1, None),
            ("fused", x1t, 65, gr2, None),
            ("fused", x2t, 65, gr3, None),
            ("two", c3, P, x24, negxx4),
        ]
        # B-stage destinations + next-block prep config per block
        #   (dsts, nxt_main, nxt_gr2x, nxt_negxx_row_tile_or_None, nxt_at_lhs, nxt_wn)
        tail_cfg = [
            ([x1t], gr2, True, wnT[1]),
            ([x2t], gr3, True, wnT[2]),
            ([c3], None, False, wnT[3]),   # writes x24/negxx4 directly
            ([c4a, c4b], None, False, None),
        ]
        xcat = [None, c3, c4a, c4b]       # k=0 handled as two half-contractions

        def emit_final_oc(oc):
            red1 = spool.tile([P, 1], F32, tag="red1")
            nc.vector.tensor_reduce(out=red1[:], in_=redv[:, oc * 6:(oc + 1) * 6],
                                    axis=mybir.AxisListType.X, op=mybir.AluOpType.max)
            reds = spool.tile([P, 1], F32, tag="reds")
            nc.vector.tensor_scalar_mul(reds[:], red1[:], SLOPE)
            nc.vector.tensor_tensor(out=red1[:], in0=red1[:], in1=reds[:],
                                    op=mybir.AluOpType.max)
            nc.sync.dma_start(out=out_d[oc * P:(oc + 1) * P, :], in_=red1[:])

        st = {}   # keyed (bi, i)

        def emit_a1(bi2, i, qs=(0, 1, 2, 3)):
            mode, glhs, grows, grhs, gneg = gram_cfg[bi2]
            key = (bi2, i)
            if key not in st:
                st[key] = {"s_sb": tkpool.tile([P, N], F32, tag="s_sb",
                                               name=f"s_sb_{bi2}_{i}")}
            s_sb = st[key]["s_sb"]
            for q in qs:
                sl = slice(q * 512, (q + 1) * 512)
                ph = pp_s.tile([P, 512], F32, space="PSUM", tag="ph")
                if mode == "fused":
                    nc.tensor.matmul(out=ph[:], lhsT=glhs[:grows, i * P:(i + 1) * P],
                                     rhs=grhs[:, sl], start=True, stop=True)
                else:
                    nc.tensor.matmul(out=ph[:], lhsT=glhs[:grows, i * P:(i + 1) * P],
                                     rhs=grhs[:, sl], start=True, stop=False)
                    nc.tensor.matmul(out=ph[:], lhsT=ones_row[:, :P],
                                     rhs=gneg[:, sl], start=False, stop=True)
                nc.scalar.copy(out=s_sb[:, sl], in_=ph[:])

        for bi, (C, O) in enumerate(BLOCKS):
            gq = []   # pending global-conv (slot, cols, oc) jobs

            def emit_a2(i, bi=bi, O=O):
                # topk rounds interleaved with the gather waves so the Pool
                # engine starts each chunk's gathers ~4.5 us earlier
                s_sb = st[(bi, i)]["s_sb"]
                v8 = tkpool.tile([P, 8], F32, tag="v8")
                # one index tile per wave: a shared tile would WAR-chain the
                # next round's max_index behind this wave's gather DMAs
                iw = [gpool.tile([P, 8], U32, tag=f"iw{r}", name=f"iw{r}")
                      for r in range(3)]
                gath = gpool.tile([P, KNN, O], F32, tag="gath")
                # rank-1 is always the point itself (s[p,p]-s[p,m]=|x_p-x_m|^2
                # >= 0; near-ties are feature-duplicates with equal A rows), so
                # slot 0 is a contiguous chunk read off the Pool engine, with
                # no dependency on the scans at all
                nc.sync.dma_start(out=gath[:, 0, :],
                                  in_=at_dram[bi][i * P:(i + 1) * P, :])
                waves = [(1, 1, 7), (8, 0, 8), (16, 0, 4)]
                for r, (j0, w0, nj) in enumerate(waves):
                    if r == 0 and i == 0:
                        # block start: scan the columns that landed early
                        # (pre-emitted gram quarters) before the last quarter
                        # arrives; top-8 of the halves' top-8s is exact
                        va = tkpool.tile([P, 16], F32, tag="va")
                        nc.vector.max(out=va[:, 0:8], in_=s_sb[:, 0:1536])
                        nc.vector.max(out=va[:, 8:16], in_=s_sb[:, 1536:2048])
                        nc.vector.max(out=v8[:], in_=va[:])
                    else:
                        nc.vector.max(out=v8[:], in_=s_sb[:])
                    nc.vector.max_index(out=iw[r][:], in_max=v8[:], in_values=s_sb[:])
                    for j in range(nj):
                        nc.gpsimd.indirect_dma_start(
                            out=gath[:, j0 + j, :], out_offset=None,
                            in_=at_dram[bi][:],
                            in_offset=IndirectOffsetOnAxis(
                                ap=iw[r][:, w0 + j:w0 + j + 1], axis=0))
                    if r < 2:
                        nc.vector.match_replace(out=s_sb[:], in_to_replace=v8[:],
                                                in_values=s_sb[:], imm_value=NEG)
                st[(bi, i)]["gath"] = gath

            def emit_b(i, bi=bi, C=C, O=O):
                gath = st[(bi, i)].pop("gath")
                mx = spool.tile([P, O], F32, tag="mx")
                if i >= NCHUNK - 2 or i <= 0:
                    # fill/drain chunks: reduce waves A+B first (they complete
                    # earlier), so only the small wave-C part sits behind the
                    # last DMAs while Pool works through its backlog
                    mab = spool.tile([P, O], F32, tag="xs", name="mab")
                    nc.vector.tensor_reduce(
                        out=mab[:], in_=gath[:, :16, :].rearrange("p k o -> p o k"),
                        axis=mybir.AxisListType.X, op=mybir.AluOpType.max)
                    nc.vector.tensor_reduce(
                        out=mx[:], in_=gath[:, 16:, :].rearrange("p k o -> p o k"),
                        axis=mybir.AxisListType.X, op=mybir.AluOpType.max)
                    nc.vector.tensor_tensor(out=mx[:], in0=mx[:], in1=mab[:],
                                            op=mybir.AluOpType.max)
                else:
                    nc.vector.tensor_reduce(
                        out=mx[:], in_=gath[:].rearrange("p k o -> p o k"),
                        axis=mybir.AxisListType.X, op=mybir.AluOpType.max)
                bt = pp_m.tile([P, O], F32, space="PSUM", tag="mm")
                nc.tensor.matmul(out=bt[:], lhsT=gram_cfg[bi][1][:C, i * P:(i + 1) * P],
                                 rhs=wdT[bi][:], start=True, stop=True)
                xt = spool.tile([P, O], F32, tag="xt")
                nc.vector.tensor_add(out=xt[:], in0=mx[:], in1=bt[:])
                xs = spool.tile([P, O], F32, tag="xs")
                nc.vector.tensor_scalar_mul(xs[:], xt[:], SLOPE)
                nc.vector.tensor_tensor(out=xt[:], in0=xt[:], in1=xs[:],
                                        op=mybir.AluOpType.max)

                dsts, nxt_gr, gr_dma, nxt_wn = tail_cfg[bi]
                cols = slice(i * P, (i + 1) * P)
                for q in range((O + P - 1) // P):
                    osz = min(P, O - q * P)
                    tp = pp_m.tile([P, P], F32, space="PSUM", tag="tp")
                    nc.tensor.transpose(out=tp[:osz, :], in_=xt[:, q * P:q * P + osz],
                                        identity=ident[:])
                    nc.scalar.copy(out=dsts[q][0:osz, cols], in_=tp[:osz, :])
                    if bi <= 2:
                        # next-block prep, fused on this chunk's columns
                        tgt2x = nxt_gr[0:osz, cols] if bi <= 1 else x24[0:osz, cols]
                        nc.scalar.activation(out=tgt2x, in_=tp[:osz, :],
                                             func=ACT.Copy, scale=2.0)
                        sqt = spool.tile([O, P], F32, tag="sqt")
                        nc.scalar.activation(out=sqt[:osz, :], in_=tp[:osz, :],
                                             func=ACT.Square)
                if bi <= 2:
                    # write the row-sum at psum partition 64 so the Act copy
                    # lands it directly in gr{2,3} row 64 (no DMA hop on the
                    # block-transition critical chain)
                    pn = pp_m.tile([65, P], F32, space="PSUM", tag="tp")
                    row = 64 if bi <= 1 else 0
                    nc.tensor.matmul(out=pn[row:row + 1, :], lhsT=ones_col[:O, :],
                                     rhs=sqt[:], start=True, stop=True)
                    if bi <= 1:
                        nc.scalar.activation(out=nxt_gr[64:65, cols],
                                             in_=pn[64:65, :], func=ACT.Copy,
                                             scale=-1.0)
                    else:
                        nc.scalar.activation(out=negxx4[:, cols], in_=pn[0:1, :],
                                             func=ACT.Copy, scale=-1.0)
                    # next-block A^T table chunk
                    O2 = BLOCKS[bi + 1][1]
                    pa = pp_m.tile([P, O2], F32, space="PSUM", tag="mm")
                    nc.tensor.matmul(out=pa[:], lhsT=dsts[0][0:O, cols], rhs=nxt_wn[:],
                                     start=True, stop=True)
                    at_sb = spool.tile([P, O2], F32, tag="at_sb")
                    nc.scalar.copy(out=at_sb[:], in_=pa[:])
                    nc.sync.dma_start(out=at_dram[bi + 1][cols, :], in_=at_sb[:])
                if bi == 3:
                    # global-conv pieces as their columns complete, dripped a
                    # few ocs per tail stage: a full 8-oc burst is ~34 us of
                    # PE matmuls that would head-block the next chunk's gram
                    if i in (3, 7, 11):
                        nq = i // 4
                        gq.extend((nq, slice(512 * nq, 512 * (nq + 1)), oc)
                                  for oc in range(8))
                    elif i == 13:
                        gq.extend((3, slice(1536, 1792), oc) for oc in range(8))
                    elif i == 14:
                        gq.extend((4, slice(1792, 1920), oc) for oc in range(8))
                    elif i == 15:
                        gq.extend((5, slice(1920, 2048), oc) for oc in range(8))
                    ndrain = 2 if i < 13 else len(gq)
                    for slot, ncols, oc in gq[:ndrain]:
                        emit_gpiece_oc(slot, ncols, oc)
                        if slot == 5:
                            emit_final_oc(oc)
                    del gq[:ndrain]

            def emit_gpiece_oc(slot, ncols, oc):
                nn = ncols.stop - ncols.start
                ocs = slice(oc * P, (oc + 1) * P)
                pg = pp_s.tile([P, 512], F32, space="PSUM", tag="ph")
                nc.tensor.matmul(out=pg[:, :nn], lhsT=wgT[0][0:64, ocs],
                                 rhs=x1t[0:64, ncols], start=True, stop=False)
                nc.tensor.matmul(out=pg[:, :nn], lhsT=wg0b[:, ocs],
                                 rhs=x2t[0:64, ncols], start=False, stop=False)
                for k in range(1, 4):
                    nc.tensor.matmul(out=pg[:, :nn], lhsT=wgT[k][:, ocs],
                                     rhs=xcat[k][:, ncols],
                                     start=False, stop=(k == 3))
                nc.vector.tensor_reduce(out=redv[:, oc * 6 + slot:oc * 6 + slot + 1],
                                        in_=pg[:, :nn], axis=mybir.AxisListType.X,
                                        op=mybir.AluOpType.max)

            if bi == 0:
                emit_a1(0, 0)
                emit_at0_prep()
            else:
                emit_a1(bi, 0, qs=(3,))   # quarters 0-2 pre-emitted by block bi-1
            for i in range(NCHUNK):
                if i + 1 < NCHUNK:
                    emit_a1(bi, i + 1)
                    if i + 1 == NCHUNK - 1 and bi < 3:
                        # next block's first gram, for the columns already
                        # complete (quarters 0-2 <- chunks 0..11): keeps PE fed
                        # through the gather drain at the block boundary
                        emit_a1(bi + 1, 0, qs=(0, 1, 2))
                emit_a2(i)
                # B(0) deferred one extra iteration: during the pipeline fill
                # Pool is still backlogged, and the wait before reduce(0)
                # would head-block the chunk-3 scans on the in-order DVE
                if i == 3:
                    emit_b(0)
                elif i == 4:
                    emit_b(1)
                    emit_b(2)
                elif i >= 5:
                    emit_b(i - 2)
            emit_b(NCHUNK - 2)
            emit_b(NCHUNK - 1)



    nc.compile()
    return nc


_NC_CACHE = None


def _get_nc():
    global _NC_CACHE
    if _NC_CACHE is None:
        _NC_CACHE = build_nc()
    return _NC_CACHE


def host_weights(W1, W2, W3, W4, Wg):
    ws = {}
    for bi, (wm, (c, o)) in enumerate(zip([W1, W2, W3, W4], BLOCKS)):
        wm = np.asarray(wm, dtype=np.float32)
        wn = wm[:, :c]
        wd = wm[:, c:] - wn
        ws[f"wnt{bi}"] = np.ascontiguousarray(wn.T)
        ws[f"wdt{bi}"] = np.ascontiguousarray(wd.T)
    ws["wgt"] = np.ascontiguousarray(np.asarray(Wg, dtype=np.float32).T)
    ws["ident"] = np.eye(P, dtype=np.float32)
    return ws


def kernel(x, W1, W2, W3, W4, Wg):
    from concourse.bass_utils import run_bass_kernel_spmd

    nc = _get_nc()
    x = np.asarray(x, dtype=np.float32)
    ws = host_weights(W1, W2, W3, W4, Wg)
    in_maps = [{"x": np.ascontiguousarray(x[b]), **ws} for b in range(B)]
    res = run_bass_kernel_spmd(nc, in_maps, core_ids=list(range(B)))
    outs = res.results if hasattr(res, "results") else res
    return np.stack([outs[b]["out"].reshape(1024) for b in range(B)], axis=0)
